# Initial kernel scaffold
#
"""ASSANet decoder (PointNet++ FP modules x4) on 8 Trainium2 NeuronCores.

Sharding: pure data-parallel over batch (B=8, one batch per core).

Per stage (coarse->fine), per core:
  P1: s = 2*u.k - |k|^2 per 128-unknown block via one bf16-split matmul
      (K=14 rows: exact to ~1e-7), then DVE max8 (top-8 values of s =
      8 nearest) + max_index (their column indices).
  P2: batched weight math (dist/recip/normalize) + gather-index reformat.
  P3: gT = kn_feats^T @ W1a^T computed directly on PE (m-part layout),
      drained bf16 to a DRAM scratch for row-gathers.
  P4: dma_gather rows of gT (3 neighbors per point, point-major), then
      fold the interpolation weights in as diagonal matmuls accumulating
      into the conv1 PSUM together with the skip-branch (W1b @ unk) matmuls;
      fused BN+ReLU on ACT.
  P5: conv2 matmuls + fused BN+ReLU.

Host side precomputes: BN folding into weights, weight transposes + bf16
casts, bf16 hi/lo splits of coordinates (lhsT/rhs for the distance matmul),
|u|^2, identity-x3 tile, and stage-A's gT (inputs only).
"""

import numpy as np
import ml_dtypes

import concourse.bass as bass
import concourse.mybir as mybir
import concourse.tile as tile
from concourse.bass_utils import run_bass_kernel_spmd

F32 = mybir.dt.float32
BF16 = mybir.dt.bfloat16
U16 = mybir.dt.uint16
I16 = mybir.dt.int16
AF = mybir.ActivationFunctionType

B = 8
BN_EPS = 1e-5

# stage configs coarse->fine; MLP = [C2 + C1, Cout, Cout]
STAGES = [
    dict(n=128,  m=32,   C1=512, C2=1024, Co=512),  # params[3]
    dict(n=512,  m=128,  C1=256, C2=512,  Co=512),  # params[2]
    dict(n=2048, m=512,  C1=128, C2=512,  Co=256),  # params[1]
    dict(n=8192, m=2048, C1=64,  C2=256,  Co=128),  # params[0]
]
# gather-source channel count per stage (stage 0 gathers host gT of W1a@feat4)
GSRC = [512, 512, 256, 128]  # == Co of that stage

NCHUNK = 512          # conv column chunk (1 PSUM bank)
GCHUNK = 2048         # dma_gather index chunk

SEL_FROM_PSUM = True  # max8/max_index read s straight from PSUM


def bf(x):
    return np.asarray(x).astype(ml_dtypes.bfloat16)


def cdiv(a, b):
    return (a + b - 1) // b


def build_module():
    nc = bass.Bass()
    d = {}

    def inp(name, shape, dt):
        d[name] = nc.declare_dram_parameter(name, list(shape), dt, isOutput=False)
        return d[name]

    for si, st in enumerate(STAGES):
        n, m, C1, Co = st["n"], st["m"], st["C1"], st["Co"]
        inp(f"lhsT{si}", (14, n), BF16)      # [uh,ul,uh,ul,1,1] rows
        inp(f"rhs{si}", (14, m), BF16)       # [2kh,2kh,2kl,2kl,-qh,-ql] rows
        inp(f"usq{si}", (128, n // 128), F32)
        inp(f"unk{si}", (C1, n), BF16)
        inp(f"w1b{si}", (C1, st["Co"]), BF16)   # W1b^T
        inp(f"b1{si}", (st["Co"], 1), F32)
        inp(f"w2{si}", (st["Co"], st["Co"]), BF16)  # W2^T
        inp(f"b2{si}", (st["Co"], 1), F32)
        if si > 0:
            inp(f"w1a{si}", (STAGES[si - 1]["Co"], st["Co"]), BF16)  # W1a^T
    inp("gt0", (STAGES[0]["m"], GSRC[0]), BF16)  # host gT for stage A
    inp("ident3", (128, 384), BF16)              # [I | I | I]
    out = nc.declare_dram_parameter("out", [128, 8192], F32, isOutput=True)

    gt_dram = [None] + [
        nc.dram_tensor(f"gtd{si}", [STAGES[si]["m"], GSRC[si]], BF16)
        for si in range(1, 4)
    ]

    with tile.TileContext(nc) as tc:
        with tc.tile_pool(name="const", bufs=1) as cpool:
            i3 = cpool.tile([128, 384], BF16)
            nc.sync.dma_start(i3[:], d["ident3"][:])

            feats_prev = None  # bf16 SBUF tiles list (128-part chunks) of prev stage out
            for si, st in enumerate(STAGES):
                feats_prev = build_stage(nc, tc, d, st, si, feats_prev,
                                         gt_dram[si], i3, out)
    return nc


def build_stage(nc, tc, d, st, si, feats_prev, gtd, i3, out_dram):
    n, m, C1, C2, Co = st["n"], st["m"], st["C1"], st["C2"], st["Co"]
    nblk = n // 128
    coutc = Co // 128          # cout chunks
    gsrc = GSRC[si]            # channels gathered per point
    nchunk = min(NCHUNK, n)
    gchunk = min(GCHUNK, n)

    import contextlib
    ctx = contextlib.ExitStack()
    with ctx:
        sb = ctx.enter_context(tc.tile_pool(name=f"sb{si}", bufs=1))
        # ---------- load stage inputs ----------
        lhsT = sb.tile([14, n], BF16)
        nc.sync.dma_start(lhsT[:], d[f"lhsT{si}"][:])
        rhs = sb.tile([14, m], BF16)
        nc.sync.dma_start(rhs[:], d[f"rhs{si}"][:])
        usq = sb.tile([128, nblk], F32)
        nc.sync.dma_start(usq[:], d[f"usq{si}"][:])
        unk = sb.tile([C1, n], BF16)
        nc.sync.dma_start(unk[:], d[f"unk{si}"][:])
        w1b = sb.tile([C1, Co], BF16)
        nc.sync.dma_start(w1b[:], d[f"w1b{si}"][:])
        w2 = sb.tile([Co, Co], BF16)
        nc.sync.dma_start(w2[:], d[f"w2{si}"][:])
        b1 = sb.tile([Co, 1], F32)
        nc.sync.dma_start(b1[:], d[f"b1{si}"][:])
        b2 = sb.tile([Co, 1], F32)
        nc.sync.dma_start(b2[:], d[f"b2{si}"][:])

        # ---------- P3: gather source gT -> DRAM ----------
        if si == 0:
            gt_src = d["gt0"]
        else:
            w1a = sb.tile([C2, Co], BF16)
            nc.sync.dma_start(w1a[:], d[f"w1a{si}"][:])
            mblk = cdiv(m, 128)
            gtsb = sb.tile([128, mblk, gsrc], BF16)
            with tc.tile_pool(name=f"gtp{si}", bufs=2, space="PSUM") as gtp:
                for mc in range(mblk):
                    mw = min(128, m - mc * 128)
                    gps = gtp.tile([128, gsrc], F32, tag="gt")
                    for kc in range(C2 // 128):
                        # lhsT = kn chunk (128 ch, mw pts), rhs = W1a^T chunk
                        nc.tensor.matmul(
                            gps[:mw, :],
                            feats_prev[kc][:, mc * 128:mc * 128 + mw],
                            w1a[kc * 128:(kc + 1) * 128, :],
                            start=(kc == 0), stop=(kc == C2 // 128 - 1))
                    nc.scalar.copy(gtsb[:mw, mc, :], gps[:mw, :])
            nc.sync.dma_start(
                gtd[:].rearrange("(b p) c -> p b c", p=128) if mblk > 1
                else gtd[:].rearrange("p c -> p 1 c"),
                gtsb[:m if mblk == 1 else 128, :, :])
            gt_src = gtd

        # ---------- P1: distance matmul + top-3 selection ----------
        vall = sb.tile([128, nblk, 8], F32)
        iall = sb.tile([128, nblk, 8], U16)
        with tc.tile_pool(name=f"sp{si}", bufs=2, space="PSUM") as spool, \
             tc.tile_pool(name=f"ssb{si}", bufs=2) as ssb:
            for b in range(nblk):
                s = spool.tile([128, m], F32, tag="s")
                nc.tensor.matmul(s[:], lhsT[:, b * 128:(b + 1) * 128], rhs[:],
                                 start=True, stop=True)
                if SEL_FROM_PSUM:
                    sv = s
                else:
                    sv = ssb.tile([128, m], F32, tag="sv")
                    nc.scalar.copy(sv[:], s[:])
                nc.vector.max(vall[:, b, :], sv[:])
                nc.vector.max_index(iall[:, b, :], vall[:, b, :], sv[:])

        # ---------- P2: weights + gather index lists ----------
        # d2 = usq - v  (clip >= 0), dist = sqrt, recip = 1/(dist+1e-8), norm
        w3 = sb.tile([128, nblk, 3], F32)
        t0 = sb.tile([128, nblk, 3], F32)
        nc.vector.tensor_sub(t0[:], usq[:].to_broadcast([128, nblk, 3]),
                             vall[:, :, 0:3])
        nc.vector.tensor_scalar_max(t0[:], t0[:], 0.0)
        nc.scalar.activation(t0[:], t0[:], AF.Sqrt)
        nc.vector.tensor_scalar_add(t0[:], t0[:], 1e-8)
        nc.vector.reciprocal(t0[:], t0[:])
        rsum = sb.tile([128, nblk], F32)
        nc.vector.reduce_sum(rsum[:], t0[:], axis=mybir.AxisListType.X)
        nc.vector.reciprocal(rsum[:], rsum[:])
        nc.vector.tensor_mult(w3[:], t0[:], rsum[:].to_broadcast([128, nblk, 3]))

        # gather index lists, one per neighbor slot k: wrapped (16, n/16) i16,
        # replicated to 128 partitions.
        idxw = []
        for k in range(3):
            t = sb.tile([128, n // 16], U16, tag=f"idxw{si}_{k}")
            for r in range(min(8, nblk * 8)):
                if r >= 8:
                    break
            for r in range(8):
                if nblk * 128 <= r * 16:
                    break
                # dst[p16, b*8 + r] = iall[r*16 + p16, b, k]
                nc.vector.tensor_copy(
                    t[0:16, r::8] if nblk > 0 else None,
                    iall[r * 16:r * 16 + 16, :, k])
            # replicate partitions 0:16 -> all 128
            nc.vector.tensor_copy(t[16:32, :], t[0:16, :])
            nc.vector.tensor_copy(t[32:64, :], t[0:32, :])
            nc.vector.tensor_copy(t[64:128, :], t[0:64, :])
            idxw.append(t)

        # ---------- P4+P5: gather, conv1, conv2 ----------
        x2 = [sb.tile([128, n], BF16, tag=f"x2_{si}_{c}") for c in range(coutc)]
        fo = [sb.tile([128, n], BF16 if si < 3 else F32, tag=f"fo{si}_{c}")
              for c in range(coutc)]

        with tc.tile_pool(name=f"gp{si}", bufs=2) as gpool, \
             tc.tile_pool(name=f"c1p{si}", bufs=2, space="PSUM") as c1p, \
             tc.tile_pool(name=f"c2p{si}", bufs=2, space="PSUM") as c2p, \
             tc.tile_pool(name=f"dgp{si}", bufs=2) as dgp:
            for gc in range(cdiv(n, gchunk)):
                g0 = gc * gchunk
                gw = min(gchunk, n - g0)
                gblk = gw // 128
                gk = []
                for k in range(3):
                    gt_tile = gpool.tile([128, gblk, gsrc], BF16, tag=f"g{k}")
                    nc.gpsimd.dma_gather(
                        out_ap=gt_tile[:],
                        in_ap=gt_src[:],
                        idxs_ap=idxw[k][:, g0 // 16:(g0 + gw) // 16],
                        num_idxs=gw,
                        num_idxs_reg=gw,
                        elem_size=gsrc,
                    )
                    gk.append(gt_tile)
                for cc in range(gw // nchunk):
                    c0 = g0 + cc * nchunk
                    cblk = nchunk // 128
                    # diag tiles for this conv chunk: (128, cblk, 3, 128)
                    dg = dgp.tile([128, cblk, 3, 128], BF16, tag="diag")
                    nc.vector.tensor_mult(
                        dg[:],
                        i3[:].to_broadcast([128, 384, cblk]).rearrange(
                            "p f b -> p b f").rearrange(
                            "p b (k c) -> p b k c", k=3),
                        w3[:, c0 // 128:c0 // 128 + cblk, :].to_broadcast(
                            [128, cblk, 3, 128]))
                    for oc in range(coutc):
                        ps = c1p.tile([128, nchunk], F32, tag="c1")
                        first = True
                        for lb in range(cblk):
                            for k in range(3):
                                nc.tensor.matmul(
                                    ps[:, lb * 128:(lb + 1) * 128],
                                    gk[cc * (nchunk // 128) // 1 * 0 + (cc * nchunk // 128 + lb - g0 // 128) if False else gk[k][:, (c0 - g0) // 128 + lb, oc * 128:(oc + 1) * 128]] if False else gk[k][:, (c0 - g0) // 128 + lb, oc * 128:(oc + 1) * 128],
                                    dg[:, lb, k, :],
                                    start=first, stop=False)
                                first = False
                        nck = cdiv(C1, 128)
                        for kc in range(nck):
                            kw = min(128, C1 - kc * 128)
                            nc.tensor.matmul(
                                ps[:],
                                w1b[kc * 128:kc * 128 + kw, oc * 128:(oc + 1) * 128],
                                unk[kc * 128:kc * 128 + kw, c0:c0 + nchunk],
                                start=False, stop=(kc == nck - 1))
                        nc.scalar.activation(
                            x2[oc][:, c0:c0 + nchunk], ps[:], AF.Relu,
                            bias=b1[oc * 128:(oc + 1) * 128, :],
                            scale=1.0)
                    # conv2 for this chunk
                    for oc in range(coutc):
                        ps2 = c2p.tile([128, nchunk], F32, tag="c2")
                        for kc in range(coutc):
                            nc.tensor.matmul(
                                ps2[:],
                                w2[kc * 128:(kc + 1) * 128, oc * 128:(oc + 1) * 128],
                                x2[kc][:, c0:c0 + nchunk],
                                start=(kc == 0), stop=(kc == coutc - 1))
                        nc.scalar.activation(
                            fo[oc][:, c0:c0 + nchunk], ps2[:], AF.Relu,
                            bias=b2[oc * 128:(oc + 1) * 128, :],
                            scale=1.0)

        if si == 3:
            nc.sync.dma_start(out_dram[:], fo[0][:])
            return None
        # keep feats for next stage (escape this stage's pool ctx via parent pool)
        return fo


# ---------------------------------------------------------------------------
# host-side prep
# ---------------------------------------------------------------------------

def _split_bf16(x):
    h = bf(x).astype(np.float32)
    l = bf(x - h).astype(np.float32)
    return h, l


def _prep_core(b, xyz, feats, params):
    """Build the per-core input map for batch b."""
    im = {}
    for si, st in enumerate(STAGES):
        n, m, C1 = st["n"], st["m"], st["C1"]
        lvl_u = 3 - si          # unknown level index (fine side of this stage)
        lvl_k = 4 - si          # known level
        u = xyz[lvl_u][b]       # (n, 3)
        k = xyz[lvl_k][b]       # (m, 3)
        uh, ul = _split_bf16(u)
        kh, kl = _split_bf16(k)
        q = np.sum(k.astype(np.float64) * k.astype(np.float64), -1)
        qh = bf(q).astype(np.float64)
        ql = q - qh
        lhsT = np.zeros((14, n), np.float32)
        lhsT[0:3] = uh.T
        lhsT[3:6] = ul.T
        lhsT[6:9] = uh.T
        lhsT[9:12] = ul.T
        lhsT[12] = 1.0
        lhsT[13] = 1.0
        rhs = np.zeros((14, m), np.float32)
        rhs[0:3] = 2.0 * kh.T
        rhs[3:6] = 2.0 * kh.T
        rhs[6:9] = 2.0 * kl.T
        rhs[9:12] = 2.0 * kl.T
        rhs[12] = -qh
        rhs[13] = -np.asarray(ql, np.float32)
        im[f"lhsT{si}"] = bf(lhsT)
        im[f"rhs{si}"] = bf(rhs)
        usq = np.sum(u * u, -1).astype(np.float32)  # (n,)
        im[f"usq{si}"] = usq.reshape(n // 128, 128).T.copy()
        im[f"unk{si}"] = bf(feats[lvl_u][b])

        (W1, g1, be1, mu1, va1), (W2, g2, be2, mu2, va2) = params[3 - si]
        s1 = g1 / np.sqrt(va1 + BN_EPS)
        W1f = (W1 * s1[:, None]).astype(np.float32)
        bias1 = (be1 - mu1 * s1).astype(np.float32)
        s2 = g2 / np.sqrt(va2 + BN_EPS)
        W2f = (W2 * s2[:, None]).astype(np.float32)
        bias2 = (be2 - mu2 * s2).astype(np.float32)
        C2 = st["C2"]
        im[f"w1b{si}"] = bf(W1f[:, C2:].T)    # (C1, Co)
        im[f"b1{si}"] = bias1[:, None].astype(np.float32)
        im[f"w2{si}"] = bf(W2f.T)
        im[f"b2{si}"] = bias2[:, None].astype(np.float32)
        if si > 0:
            im[f"w1a{si}"] = bf(W1f[:, :C2].T)  # (C2, Co)
        else:
            # host gT for stage A: (m, Co) = feat4^T @ W1a^T
            W1a = W1f[:, :C2]
            gt = feats[4][b].T.astype(np.float32) @ W1a.T.astype(np.float32)
            im["gt0"] = bf(gt)
    id3 = np.zeros((128, 384), np.float32)
    for kk in range(3):
        id3[:, kk * 128:(kk + 1) * 128] = np.eye(128)
    im["ident3"] = bf(id3)
    return im


_nc_cache = {}


def kernel(**inputs):
    xyz = [np.asarray(inputs[f"xyz{l}"], np.float32) for l in range(5)]
    feats = [np.asarray(inputs[f"feat{l}"], np.float32) for l in range(5)]
    params = [[(np.asarray(W, np.float32), np.asarray(g, np.float32),
                np.asarray(bb, np.float32), np.asarray(mu, np.float32),
                np.asarray(v, np.float32)) for (W, g, bb, mu, v) in lp]
              for lp in inputs["params"]]

    if "nc" not in _nc_cache:
        _nc_cache["nc"] = build_module()
    nc = _nc_cache["nc"]

    in_maps = [_prep_core(b, xyz, feats, params) for b in range(B)]
    res = run_bass_kernel_spmd(nc, in_maps, core_ids=list(range(B)))
    out = np.stack([res.results[b]["out"] for b in range(B)], 0)
    return out.astype(np.float32)


# revision 21
# speedup vs baseline: 27.6890x; 27.6890x over previous
"""ASSANet decoder (PointNet++ FP modules x4) on 8 Trainium2 NeuronCores.

Sharding: pure data-parallel over batch (B=8, one batch per core).

Per stage (coarse->fine), per core:
  P3: gT = kn_feats^T @ W1a^T computed directly on PE (m-part layout),
      drained bf16 to a DRAM scratch (gather source).
  P1: s = 2*u.k - |k|^2 per 128-unknown block via one bf16-split matmul
      (K=14 rows, exact to ~1e-7); DVE max8 (top-8 s values = 8 nearest)
      + max_index (their column indices).
  P2: batched weight math (dist/recip/normalize) + gather-index reformat.
  P4: dma_gather rows of gT (3 per point, point-major layout), fold the
      interpolation weights in via diagonal matmuls accumulating into the
      conv1 PSUM along with the skip-branch (W1b @ unk) matmuls; fused
      BN+ReLU on ACT.
  P5: conv2 matmuls + fused BN+ReLU.

Host precomputes: BN folding, weight transposes/bf16 casts, bf16 hi/lo
coordinate splits (lhsT/rhs of the distance matmul), |u|^2, identity-x3
tile, and stage-A's gT (pure input transforms).
"""

import contextlib
import os

import numpy as np
import ml_dtypes

import concourse.bass as bass
import concourse.bacc as bacc
import concourse.mybir as mybir
import concourse.tile as tile
from concourse.bass_utils import run_bass_kernel_spmd

F32 = mybir.dt.float32
BF16 = mybir.dt.bfloat16
U16 = mybir.dt.uint16
I16 = mybir.dt.int16
AF = mybir.ActivationFunctionType
AX = mybir.AxisListType

B = 8
BN_EPS = 1e-5
import os
STAGE_LIMIT = int(os.environ.get("STAGE_LIMIT", "4"))
ITERS = int(os.environ.get("ITERS", "1"))

# stage configs coarse->fine; MLP = [C2 + C1, Co, Co]
STAGES = [
    dict(n=128,  m=32,   C1=512, C2=1024, Co=512),  # params[3]
    dict(n=512,  m=128,  C1=256, C2=512,  Co=512),  # params[2]
    dict(n=2048, m=512,  C1=128, C2=512,  Co=256),  # params[1]
    dict(n=8192, m=2048, C1=64,  C2=256,  Co=128),  # params[0]
]
GSRC = [512, 512, 256, 128]   # gather-source channels = Co of the stage

NCHUNK = 512                  # conv column chunk (1 PSUM bank)
GCHUNK = int(os.environ.get("GCHUNK", "2048"))   # dma_gather index chunk


def bf(x):
    return np.asarray(x).astype(ml_dtypes.bfloat16)


def cdiv(a, b):
    return (a + b - 1) // b


def _load_rows(nc, pool, dram, rows, cols, dt, tag):
    """Load a (rows, cols) DRAM tensor as a list of 128-row SBUF tiles."""
    tiles = []
    for c in range(cdiv(rows, 128)):
        r0 = c * 128
        rw = min(128, rows - r0)
        t = pool.tile([rw, cols], dt, tag=f"{tag}_{c}", name=f"{tag}_{c}")
        nc.sync.dma_start(t[:], dram[r0:r0 + rw, :])
        tiles.append(t)
    return tiles


def build_module():
    nc = bacc.Bacc()
    d = {}

    def inp(name, shape, dt):
        d[name] = nc.declare_dram_parameter(name, list(shape), dt, isOutput=False)

    for si, st in enumerate(STAGES):
        n, m, C1, Co = st["n"], st["m"], st["C1"], st["Co"]
        inp(f"lhsT{si}", (14, n), BF16)
        inp(f"rhs{si}", (14, m), BF16)
        inp(f"usq{si}", (128, n // 128), F32)
        inp(f"unk{si}", (C1, n), BF16)
        inp(f"w1b{si}", (C1, Co), BF16)          # W1b^T
        inp(f"b1{si}", (Co, 1), F32)
        inp(f"w2{si}", (Co, Co), BF16)           # W2^T
        inp(f"b2{si}", (Co, 1), F32)
        if si > 0:
            inp(f"w1a{si}", (st["C2"], Co), BF16)  # W1a^T
    inp("gt0", (STAGES[0]["m"], GSRC[0]), BF16)
    if os.environ.get("DEBUG_IDX"):
        d["idxo"] = nc.declare_dram_parameter(
            "idxo", [STAGES[1]["n"], 8], mybir.dt.uint16, isOutput=True)
        d["vo"] = nc.declare_dram_parameter(
            "vo", [STAGES[1]["n"], 8], F32, isOutput=True)
    inp("ident3", (128, 384), BF16)
    out_dram = nc.declare_dram_parameter("out", [128, 8192], F32, isOutput=True)

    with tile.TileContext(nc) as tc:
        with tc.tile_pool(name="keep", bufs=1) as keep, \
             tc.tile_pool(name="dramp", bufs=1, space="DRAM") as dramp:
            gt_dram = [None] + [
                dramp.tile([STAGES[si]["m"], GSRC[si]], BF16,
                           name=f"gtd{si}", tag=f"gtd{si}")
                for si in range(1, 4)
            ]
            idx_dram = [
                dramp.tile([STAGES[si]["n"], 8], mybir.dt.uint16,
                           name=f"idxd{si}", tag=f"idxd{si}")
                for si in range(4)
            ]
            i3 = keep.tile([128, 384], BF16)
            nc.sync.dma_start(i3[:], d["ident3"][:])
            feats_prev = None
            for it in range(ITERS):
                for si, st in enumerate(STAGES[:STAGE_LIMIT]):
                    feats_prev = _build_stage(nc, tc, d, st, si, feats_prev,
                                              gt_dram[si], idx_dram[si], i3,
                                              out_dram, keep, it)
            if STAGE_LIMIT < 4:
                nw = STAGES[STAGE_LIMIT - 1]["n"]
                for oc, t in enumerate(feats_prev):
                    nc.gpsimd.dma_start(
                        out_dram[:, oc * nw:(oc + 1) * nw], t[:])
    nc.finalize()
    return nc


def _build_stage(nc, tc, d, st, si, feats_prev, gtd, idxd, i3, out_dram, keep, it=0):
    n, m, C1, C2, Co = st["n"], st["m"], st["C1"], st["C2"], st["Co"]
    nblk = n // 128
    coutc = Co // 128
    gsrc = GSRC[si]
    nchunk = min(NCHUNK, n)
    gchunk = min(GCHUNK, n)
    last = si == 3

    ctx = contextlib.ExitStack()
    with ctx:
        sb = ctx.enter_context(tc.tile_pool(name=f"sb{it}_{si}", bufs=1))

        lhsT = sb.tile([14, n], BF16, tag="lhsT")
        nc.sync.dma_start(lhsT[:], d[f"lhsT{si}"][:])
        rhs = sb.tile([14, m], BF16, tag="rhs")
        nc.sync.dma_start(rhs[:], d[f"rhs{si}"][:])
        usq = sb.tile([128, nblk], F32, tag="usq")
        nc.sync.dma_start(usq[:], d[f"usq{si}"][:])
        unk = _load_rows(nc, sb, d[f"unk{si}"], C1, n, BF16, "unk")
        w1b = _load_rows(nc, sb, d[f"w1b{si}"], C1, Co, BF16, "w1b")
        w2 = _load_rows(nc, sb, d[f"w2{si}"], Co, Co, BF16, "w2")
        b1 = _load_rows(nc, sb, d[f"b1{si}"], Co, 1, F32, "b1")
        b2 = _load_rows(nc, sb, d[f"b2{si}"], Co, 1, F32, "b2")

        # ---------- P3: gather source gT (m-part, gsrc) -> DRAM ----------
        if si == 0:
            gt_src = d["gt0"]
        else:
            w1a = _load_rows(nc, sb, d[f"w1a{si}"], C2, Co, BF16, "w1a")
            mblk = m // 128
            gtsb = sb.tile([128, mblk, gsrc], BF16, tag="gtsb")
            kc_n = C2 // 128
            with tc.tile_pool(name=f"gtp{it}_{si}", bufs=2, space="PSUM") as gtp:
                for mc in range(mblk):
                    gps = gtp.tile([128, gsrc], F32, tag="gt")
                    for kc in range(kc_n):
                        nc.tensor.matmul(
                            gps[:],
                            feats_prev[kc][:, mc * 128:(mc + 1) * 128],
                            w1a[kc][:],
                            start=(kc == 0), stop=(kc == kc_n - 1))
                    nc.scalar.copy(gtsb[:, mc, :], gps[:])
            nc.sync.dma_start(
                gtd[:].rearrange("(b p) c -> p b c", p=128), gtsb[:])
            gt_src = gtd
            if os.environ.get("DEBUG_GT") and si == 1:
                nc.gpsimd.dma_start(out_dram[:, 4096:4096 + gsrc],
                                    gtsb[:, 0, :])

        # ---------- P1: distance matmul + top-8 selection ----------
        vall = sb.tile([128, nblk, 8], F32, tag="vall")
        iall = sb.tile([128, nblk, 8], U16, tag="iall")
        with tc.tile_pool(name=f"sp{it}_{si}", bufs=2, space="PSUM") as spool:
            for b in range(nblk):
                s = spool.tile([128, m], F32, tag="s")
                for ms in range(0, m, 512):
                    mw = min(512, m - ms)
                    nc.tensor.matmul(s[:, ms:ms + mw],
                                     lhsT[:, b * 128:(b + 1) * 128],
                                     rhs[:, ms:ms + mw],
                                     start=True, stop=True)
                nc.vector.max(vall[:, b, :], s[:])
                nc.vector.max_index(iall[:, b, :], vall[:, b, :], s[:])

        if os.environ.get("DEBUG_IDX") and si == 1:
            nc.sync.dma_start(d["idxo"][:].rearrange("(b p) k -> p b k", p=128),
                              iall[:])
            nc.sync.dma_start(d["vo"][:].rearrange("(b p) k -> p b k", p=128),
                              vall[:])
        # ---------- P2: interpolation weights + gather index lists ----------
        w3 = sb.tile([128, nblk, 3], F32, tag="w3")
        t0 = sb.tile([128, nblk, 3], F32, tag="t0")
        t1 = sb.tile([128, nblk, 3], F32, tag="t1")
        nc.vector.tensor_sub(t0[:], usq[:].to_broadcast([128, nblk, 3]),
                             vall[:, :, 0:3])
        nc.vector.tensor_scalar_max(t0[:], t0[:], 0.0)
        nc.scalar.activation(t1[:], t0[:], AF.Sqrt)
        nc.vector.tensor_scalar_add(t1[:], t1[:], 1e-8)
        nc.vector.reciprocal(t0[:], t1[:])
        rsum = sb.tile([128, nblk], F32, tag="rsum")
        nc.vector.reduce_sum(rsum[:], t0[:], axis=AX.X)
        nc.vector.reciprocal(rsum[:], rsum[:])
        nc.vector.tensor_mul(w3[:], t0[:], rsum[:].to_broadcast([128, nblk, 3]))

        # round-trip through DRAM to rewrap indices: flat point i lives at
        # partition i%16, column i//16 (dma_gather's expected layout),
        # replicated to all 128 partitions.
        nc.sync.dma_start(idxd[:].rearrange("(b p) k -> p b k", p=128), iall[:])
        wr = idxd[:].rearrange("(s p) k -> p s k", p=16)
        idxw = []
        for k in range(3):
            t = sb.tile([128, n // 16], U16, tag=f"idxw{k}", name=f"idxw{si}_{k}")
            nc.sync.dma_start(t[0:16, :], wr[:, :, k])
            nc.sync.dma_start(t[16:32, :], t[0:16, :])
            nc.sync.dma_start(t[32:64, :], t[0:32, :])
            nc.sync.dma_start(t[64:128, :], t[0:64, :])
            idxw.append(t)

        # ---------- P4 + P5: gather, conv1, conv2 ----------
        x2 = [sb.tile([128, n], BF16, tag=f"x2_{c}", name=f"x2_{si}_{c}")
              for c in range(coutc)]
        fo = [keep.tile([128, n], F32 if last else BF16, tag=f"fo{si}_{c}",
                        name=f"fo{it}_{si}_{c}")
              for c in range(coutc)]

        c1kc = cdiv(C1, 128)
        with tc.tile_pool(name=f"gp{it}_{si}", bufs=2) as gpool, \
             tc.tile_pool(name=f"c1p{it}_{si}", bufs=2, space="PSUM") as c1p, \
             tc.tile_pool(name=f"c2p{it}_{si}", bufs=2, space="PSUM") as c2p, \
             tc.tile_pool(name=f"dgp{it}_{si}", bufs=2) as dgp:
            for gc in range(cdiv(n, gchunk)):
                g0 = gc * gchunk
                gw = min(gchunk, n - g0)
                gblk = gw // 128
                gk = []
                for k in range(3):
                    gt_tile = gpool.tile([128, gblk, gsrc], BF16, tag=f"g{k}")
                    nc.gpsimd.dma_gather(
                        out_ap=gt_tile[:],
                        in_ap=gt_src[:],
                        idxs_ap=idxw[k][:, g0 // 16:(g0 + gw) // 16].bitcast(I16),
                        num_idxs=gw,
                        num_idxs_reg=gw,
                        elem_size=gsrc,
                    )
                    gk.append(gt_tile)
                for cc in range(gw // nchunk):
                    c0 = g0 + cc * nchunk
                    cblk = nchunk // 128
                    dg = dgp.tile([128, cblk, 3, 128], BF16, tag="diag")
                    for lb in range(cblk):
                        for k in range(3):
                            nc.vector.tensor_scalar_mul(
                                dg[:, lb, k], i3[:, k * 128:(k + 1) * 128],
                                w3[:, c0 // 128 + lb, k:k + 1])
                    for oc in range(coutc):
                        ps = c1p.tile([128, nchunk], F32, tag="c1")
                        for kc in range(c1kc):
                            nc.tensor.matmul(
                                ps[:],
                                w1b[kc][:, oc * 128:(oc + 1) * 128],
                                unk[kc][:, c0:c0 + nchunk],
                                start=(kc == 0), stop=False,
                                skip_group_check=True)
                        for lb in range(cblk):
                            for k in range(3):
                                nc.tensor.matmul(
                                    ps[:, lb * 128:(lb + 1) * 128],
                                    gk[k][:, (c0 - g0) // 128 + lb,
                                          oc * 128:(oc + 1) * 128],
                                    dg[:, lb, k, :],
                                    start=False,
                                    stop=(lb == cblk - 1 and k == 2),
                                    skip_group_check=True)
                        nc.scalar.activation(
                            x2[oc][:, c0:c0 + nchunk], ps[:], AF.Relu,
                            bias=b1[oc][:], scale=1.0)
                    for oc in range(coutc):
                        ps2 = c2p.tile([128, nchunk], F32, tag="c2")
                        for kc in range(coutc):
                            nc.tensor.matmul(
                                ps2[:],
                                w2[kc][:, oc * 128:(oc + 1) * 128],
                                x2[kc][:, c0:c0 + nchunk],
                                start=(kc == 0), stop=(kc == coutc - 1))
                        nc.scalar.activation(
                            fo[oc][:, c0:c0 + nchunk], ps2[:], AF.Relu,
                            bias=b2[oc][:], scale=1.0)

        if os.environ.get("DEBUG_X2") and si == 1:
            for oc in range(coutc):
                nc.gpsimd.dma_start(
                    out_dram[:, 4096 + oc * n:4096 + (oc + 1) * n], x2[oc][:])
        if last:
            nc.sync.dma_start(out_dram[:], fo[0][:])
            return None
        return fo


# ---------------------------------------------------------------------------
# host-side prep
# ---------------------------------------------------------------------------

def _split_bf16(x):
    h = bf(x).astype(np.float32)
    l = bf(x.astype(np.float32) - h).astype(np.float32)
    return h, l


def _prep_shared(params):
    """Per-core-invariant inputs (weights, identity)."""
    im = {}
    for si, st in enumerate(STAGES):
        C2 = st["C2"]
        (W1, g1, be1, mu1, va1), (W2, g2, be2, mu2, va2) = params[3 - si]
        s1 = g1 / np.sqrt(va1 + BN_EPS)
        W1f = (W1 * s1[:, None]).astype(np.float32)
        bias1 = (be1 - mu1 * s1).astype(np.float32)
        s2 = g2 / np.sqrt(va2 + BN_EPS)
        W2f = (W2 * s2[:, None]).astype(np.float32)
        bias2 = (be2 - mu2 * s2).astype(np.float32)
        im[f"w1b{si}"] = np.ascontiguousarray(bf(W1f[:, C2:].T))
        im[f"b1{si}"] = np.ascontiguousarray(bias1[:, None])
        im[f"w2{si}"] = np.ascontiguousarray(bf(W2f.T))
        im[f"b2{si}"] = np.ascontiguousarray(bias2[:, None])
        if si > 0:
            im[f"w1a{si}"] = np.ascontiguousarray(bf(W1f[:, :C2].T))
        else:
            im["_W1a0"] = W1f[:, :C2]  # used per-core for gt0
    id3 = np.zeros((128, 384), np.float32)
    for kk in range(3):
        id3[:, kk * 128:(kk + 1) * 128] = np.eye(128)
    im["ident3"] = bf(id3)
    return im


def _prep_core(b, xyz, feats, shared):
    im = {k: v for k, v in shared.items() if not k.startswith("_")}
    for si, st in enumerate(STAGES):
        n, m = st["n"], st["m"]
        u = xyz[3 - si][b]      # (n, 3) unknown side
        kk = xyz[4 - si][b]     # (m, 3) known side
        uh, ul = _split_bf16(u)
        kh, kl = _split_bf16(kk)
        q = np.sum(kk.astype(np.float64) ** 2, -1)
        qh = bf(q).astype(np.float64)
        ql = (q - qh).astype(np.float32)
        lhsT = np.zeros((14, n), np.float32)
        lhsT[0:3] = uh.T
        lhsT[3:6] = ul.T
        lhsT[6:9] = uh.T
        lhsT[9:12] = ul.T
        lhsT[12] = 1.0
        lhsT[13] = 1.0
        rhs = np.zeros((14, m), np.float32)
        rhs[0:3] = 2.0 * kh.T
        rhs[3:6] = 2.0 * kh.T
        rhs[6:9] = 2.0 * kl.T
        rhs[9:12] = 2.0 * kl.T
        rhs[12] = -qh.astype(np.float32)
        rhs[13] = -ql
        im[f"lhsT{si}"] = bf(lhsT)
        im[f"rhs{si}"] = bf(rhs)
        usq = np.sum(u.astype(np.float32) ** 2, -1)
        im[f"usq{si}"] = np.ascontiguousarray(usq.reshape(n // 128, 128).T)
        im[f"unk{si}"] = np.ascontiguousarray(bf(feats[3 - si][b]))
        if si == 0:
            gt = feats[4][b].T.astype(np.float32) @ shared["_W1a0"].T
            im["gt0"] = np.ascontiguousarray(bf(gt))
    return im


_cache = {}


def kernel(**inputs):
    xyz = [np.asarray(inputs[f"xyz{l}"], np.float32) for l in range(5)]
    feats = [np.asarray(inputs[f"feat{l}"], np.float32) for l in range(5)]
    params = [[(np.asarray(W, np.float32), np.asarray(g, np.float32),
                np.asarray(bb, np.float32), np.asarray(mu, np.float32),
                np.asarray(v, np.float32)) for (W, g, bb, mu, v) in lp]
              for lp in inputs["params"]]

    if "nc" not in _cache:
        _cache["nc"] = build_module()
    nc = _cache["nc"]

    shared = _prep_shared(params)
    in_maps = [_prep_core(b, xyz, feats, shared) for b in range(B)]
    res = run_bass_kernel_spmd(nc, in_maps, core_ids=list(range(B)))
    out = np.stack([res.results[b]["out"] for b in range(B)], 0)
    return out.astype(np.float32)


# revision 22
# speedup vs baseline: 28.5209x; 1.0300x over previous
"""ASSANet decoder (PointNet++ FP modules x4) on 8 Trainium2 NeuronCores.

Sharding: pure data-parallel over batch (B=8, one batch per core).

Per stage (coarse->fine), per core:
  P3: gT = kn_feats^T @ W1a^T computed directly on PE (m-part layout),
      drained bf16 to a DRAM scratch (gather source).
  P1: s = 2*u.k - |k|^2 per 128-unknown block via one bf16-split matmul
      (K=14 rows, exact to ~1e-7); DVE max8 (top-8 s values = 8 nearest)
      + max_index (their column indices).
  P2: batched weight math (dist/recip/normalize) + gather-index reformat.
  P4: dma_gather rows of gT (3 per point, point-major layout), fold the
      interpolation weights in via diagonal matmuls accumulating into the
      conv1 PSUM along with the skip-branch (W1b @ unk) matmuls; fused
      BN+ReLU on ACT.
  P5: conv2 matmuls + fused BN+ReLU.

Host precomputes: BN folding, weight transposes/bf16 casts, bf16 hi/lo
coordinate splits (lhsT/rhs of the distance matmul), |u|^2, identity-x3
tile, and stage-A's gT (pure input transforms).
"""

import contextlib
import os

import numpy as np
import ml_dtypes

import concourse.bass as bass
import concourse.bacc as bacc
import concourse.mybir as mybir
import concourse.tile as tile
from concourse.bass_utils import run_bass_kernel_spmd

F32 = mybir.dt.float32
BF16 = mybir.dt.bfloat16
U16 = mybir.dt.uint16
I16 = mybir.dt.int16
AF = mybir.ActivationFunctionType
AX = mybir.AxisListType

B = 8
BN_EPS = 1e-5
import os
STAGE_LIMIT = int(os.environ.get("STAGE_LIMIT", "4"))
ITERS = int(os.environ.get("ITERS", "1"))
SEL_SBUF = os.environ.get("SEL_SBUF", "0") == "1"
SKIP_GATHER = os.environ.get("SKIP_GATHER", "0") == "1"
SKIP_SEL = os.environ.get("SKIP_SEL", "0") == "1"

# stage configs coarse->fine; MLP = [C2 + C1, Co, Co]
STAGES = [
    dict(n=128,  m=32,   C1=512, C2=1024, Co=512),  # params[3]
    dict(n=512,  m=128,  C1=256, C2=512,  Co=512),  # params[2]
    dict(n=2048, m=512,  C1=128, C2=512,  Co=256),  # params[1]
    dict(n=8192, m=2048, C1=64,  C2=256,  Co=128),  # params[0]
]
GSRC = [512, 512, 256, 128]   # gather-source channels = Co of the stage

NCHUNK = 512                  # conv column chunk (1 PSUM bank)
GCHUNK = int(os.environ.get("GCHUNK", "2048"))   # dma_gather index chunk


def bf(x):
    return np.asarray(x).astype(ml_dtypes.bfloat16)


def cdiv(a, b):
    return (a + b - 1) // b


def _load_rows(nc, pool, dram, rows, cols, dt, tag):
    """Load a (rows, cols) DRAM tensor as a list of 128-row SBUF tiles."""
    tiles = []
    for c in range(cdiv(rows, 128)):
        r0 = c * 128
        rw = min(128, rows - r0)
        t = pool.tile([rw, cols], dt, tag=f"{tag}_{c}", name=f"{tag}_{c}")
        nc.sync.dma_start(t[:], dram[r0:r0 + rw, :])
        tiles.append(t)
    return tiles


def build_module():
    nc = bacc.Bacc()
    d = {}

    def inp(name, shape, dt):
        d[name] = nc.declare_dram_parameter(name, list(shape), dt, isOutput=False)

    for si, st in enumerate(STAGES):
        n, m, C1, Co = st["n"], st["m"], st["C1"], st["Co"]
        inp(f"lhsT{si}", (14, n), BF16)
        inp(f"rhs{si}", (14, m), BF16)
        inp(f"usq{si}", (128, n // 128), F32)
        inp(f"unk{si}", (C1, n), BF16)
        inp(f"w1b{si}", (C1, Co), BF16)          # W1b^T
        inp(f"b1{si}", (Co, 1), F32)
        inp(f"w2{si}", (Co, Co), BF16)           # W2^T
        inp(f"b2{si}", (Co, 1), F32)
        if si > 0:
            inp(f"w1a{si}", (st["C2"], Co), BF16)  # W1a^T
    inp("gt0", (STAGES[0]["m"], GSRC[0]), BF16)
    if os.environ.get("DEBUG_IDX"):
        d["idxo"] = nc.declare_dram_parameter(
            "idxo", [STAGES[1]["n"], 8], mybir.dt.uint16, isOutput=True)
        d["vo"] = nc.declare_dram_parameter(
            "vo", [STAGES[1]["n"], 8], F32, isOutput=True)
    inp("ident3", (128, 384), BF16)
    out_dram = nc.declare_dram_parameter("out", [128, 8192], F32, isOutput=True)

    with tile.TileContext(nc) as tc:
        with tc.tile_pool(name="keep", bufs=1) as keep, \
             tc.tile_pool(name="dramp", bufs=1, space="DRAM") as dramp:
            gt_dram = [None] + [
                dramp.tile([STAGES[si]["m"], GSRC[si]], BF16,
                           name=f"gtd{si}", tag=f"gtd{si}")
                for si in range(1, 4)
            ]
            idx_dram = [
                dramp.tile([STAGES[si]["n"], 8], mybir.dt.uint16,
                           name=f"idxd{si}", tag=f"idxd{si}")
                for si in range(4)
            ]
            i3 = keep.tile([128, 384], BF16)
            nc.sync.dma_start(i3[:], d["ident3"][:])
            feats_prev = None
            for it in range(ITERS):
                for si, st in enumerate(STAGES[:STAGE_LIMIT]):
                    feats_prev = _build_stage(nc, tc, d, st, si, feats_prev,
                                              gt_dram[si], idx_dram[si], i3,
                                              out_dram, keep, it)
            if STAGE_LIMIT < 4:
                nw = STAGES[STAGE_LIMIT - 1]["n"]
                for oc, t in enumerate(feats_prev):
                    nc.gpsimd.dma_start(
                        out_dram[:, oc * nw:(oc + 1) * nw], t[:])
    nc.finalize()
    return nc


def _build_stage(nc, tc, d, st, si, feats_prev, gtd, idxd, i3, out_dram, keep, it=0):
    n, m, C1, C2, Co = st["n"], st["m"], st["C1"], st["C2"], st["Co"]
    nblk = n // 128
    coutc = Co // 128
    gsrc = GSRC[si]
    nchunk = min(NCHUNK, n)
    gchunk = min(GCHUNK, n)
    last = si == 3

    ctx = contextlib.ExitStack()
    with ctx:
        sb = ctx.enter_context(tc.tile_pool(name=f"sb{it}_{si}", bufs=1))

        lhsT = sb.tile([14, n], BF16, tag="lhsT")
        nc.sync.dma_start(lhsT[:], d[f"lhsT{si}"][:])
        rhs = sb.tile([14, m], BF16, tag="rhs")
        nc.sync.dma_start(rhs[:], d[f"rhs{si}"][:])
        usq = sb.tile([128, nblk], F32, tag="usq")
        nc.sync.dma_start(usq[:], d[f"usq{si}"][:])
        unk = _load_rows(nc, sb, d[f"unk{si}"], C1, n, BF16, "unk")
        w1b = _load_rows(nc, sb, d[f"w1b{si}"], C1, Co, BF16, "w1b")
        w2 = _load_rows(nc, sb, d[f"w2{si}"], Co, Co, BF16, "w2")
        b1 = _load_rows(nc, sb, d[f"b1{si}"], Co, 1, F32, "b1")
        b2 = _load_rows(nc, sb, d[f"b2{si}"], Co, 1, F32, "b2")

        # ---------- P3: gather source gT (m-part, gsrc) -> DRAM ----------
        if si == 0:
            gt_src = d["gt0"]
        else:
            w1a = _load_rows(nc, sb, d[f"w1a{si}"], C2, Co, BF16, "w1a")
            mblk = m // 128
            gtsb = sb.tile([128, mblk, gsrc], BF16, tag="gtsb")
            kc_n = C2 // 128
            with tc.tile_pool(name=f"gtp{it}_{si}", bufs=2, space="PSUM") as gtp:
                for mc in range(mblk):
                    gps = gtp.tile([128, gsrc], F32, tag="gt")
                    for kc in range(kc_n):
                        nc.tensor.matmul(
                            gps[:],
                            feats_prev[kc][:, mc * 128:(mc + 1) * 128],
                            w1a[kc][:],
                            start=(kc == 0), stop=(kc == kc_n - 1))
                    nc.scalar.copy(gtsb[:, mc, :], gps[:])
            nc.sync.dma_start(
                gtd[:].rearrange("(b p) c -> p b c", p=128), gtsb[:])
            gt_src = gtd
            if os.environ.get("DEBUG_GT") and si == 1:
                nc.gpsimd.dma_start(out_dram[:, 4096:4096 + gsrc],
                                    gtsb[:, 0, :])

        # ---------- P1: distance matmul + top-8 selection ----------
        vall = sb.tile([128, nblk, 8], F32, tag="vall")
        iall = sb.tile([128, nblk, 8], U16, tag="iall")
        with tc.tile_pool(name=f"sp{it}_{si}", bufs=2, space="PSUM") as spool, \
             tc.tile_pool(name=f"ssb{it}_{si}", bufs=2) as ssbp:
            for b in range(nblk):
                s = spool.tile([128, m], F32, tag="s")
                for ms in range(0, m, 512):
                    mw = min(512, m - ms)
                    nc.tensor.matmul(s[:, ms:ms + mw],
                                     lhsT[:, b * 128:(b + 1) * 128],
                                     rhs[:, ms:ms + mw],
                                     start=True, stop=True)
                if SKIP_SEL:
                    if b == 0:
                        nc.vector.memset(vall[:], 1.0)
                        nc.vector.memset(iall[:], 0)
                    continue
                if SEL_SBUF:
                    sv = ssbp.tile([128, m], F32, tag="sv")
                    nc.scalar.copy(sv[:], s[:])
                else:
                    sv = s
                nc.vector.max(vall[:, b, :], sv[:])
                nc.vector.max_index(iall[:, b, :], vall[:, b, :], sv[:])

        if os.environ.get("DEBUG_IDX") and si == 1:
            nc.sync.dma_start(d["idxo"][:].rearrange("(b p) k -> p b k", p=128),
                              iall[:])
            nc.sync.dma_start(d["vo"][:].rearrange("(b p) k -> p b k", p=128),
                              vall[:])
        # ---------- P2: interpolation weights + gather index lists ----------
        w3 = sb.tile([128, nblk, 3], F32, tag="w3")
        t0 = sb.tile([128, nblk, 3], F32, tag="t0")
        t1 = sb.tile([128, nblk, 3], F32, tag="t1")
        nc.vector.tensor_sub(t0[:], usq[:].to_broadcast([128, nblk, 3]),
                             vall[:, :, 0:3])
        nc.vector.tensor_scalar_max(t0[:], t0[:], 0.0)
        nc.scalar.activation(t1[:], t0[:], AF.Sqrt)
        nc.vector.tensor_scalar_add(t1[:], t1[:], 1e-8)
        nc.vector.reciprocal(t0[:], t1[:])
        rsum = sb.tile([128, nblk], F32, tag="rsum")
        nc.vector.reduce_sum(rsum[:], t0[:], axis=AX.X)
        nc.vector.reciprocal(rsum[:], rsum[:])
        nc.vector.tensor_mul(w3[:], t0[:], rsum[:].to_broadcast([128, nblk, 3]))

        # round-trip through DRAM to rewrap indices: flat point i lives at
        # partition i%16, column i//16 (dma_gather's expected layout),
        # replicated to all 128 partitions.
        nc.sync.dma_start(idxd[:].rearrange("(b p) k -> p b k", p=128), iall[:])
        wr = idxd[:].rearrange("(s p) k -> p s k", p=16)
        idxw = []
        for k in range(3):
            t = sb.tile([128, n // 16], U16, tag=f"idxw{k}", name=f"idxw{si}_{k}")
            nc.sync.dma_start(t[0:16, :], wr[:, :, k])
            nc.sync.dma_start(t[16:32, :], t[0:16, :])
            nc.sync.dma_start(t[32:64, :], t[0:32, :])
            nc.sync.dma_start(t[64:128, :], t[0:64, :])
            idxw.append(t)

        # ---------- P4 + P5: gather, conv1, conv2 ----------
        x2 = [sb.tile([128, n], BF16, tag=f"x2_{c}", name=f"x2_{si}_{c}")
              for c in range(coutc)]
        fo = [keep.tile([128, n], F32 if last else BF16, tag=f"fo{si}_{c}",
                        name=f"fo{it}_{si}_{c}")
              for c in range(coutc)]

        c1kc = cdiv(C1, 128)
        with tc.tile_pool(name=f"gp{it}_{si}", bufs=2) as gpool, \
             tc.tile_pool(name=f"c1p{it}_{si}", bufs=2, space="PSUM") as c1p, \
             tc.tile_pool(name=f"c2p{it}_{si}", bufs=2, space="PSUM") as c2p, \
             tc.tile_pool(name=f"dgp{it}_{si}", bufs=2) as dgp:
            for gc in range(cdiv(n, gchunk)):
                g0 = gc * gchunk
                gw = min(gchunk, n - g0)
                gblk = gw // 128
                gk = []
                for k in range(3):
                    gt_tile = gpool.tile([128, gblk, gsrc], BF16, tag=f"g{k}")
                    if SKIP_GATHER:
                        gk.append(gt_tile)
                        continue
                    nc.gpsimd.dma_gather(
                        out_ap=gt_tile[:],
                        in_ap=gt_src[:],
                        idxs_ap=idxw[k][:, g0 // 16:(g0 + gw) // 16].bitcast(I16),
                        num_idxs=gw,
                        num_idxs_reg=gw,
                        elem_size=gsrc,
                    )
                    gk.append(gt_tile)
                for cc in range(gw // nchunk):
                    c0 = g0 + cc * nchunk
                    cblk = nchunk // 128
                    dg = dgp.tile([128, cblk, 3, 128], BF16, tag="diag")
                    for lb in range(cblk):
                        for k in range(3):
                            nc.vector.tensor_scalar_mul(
                                dg[:, lb, k], i3[:, k * 128:(k + 1) * 128],
                                w3[:, c0 // 128 + lb, k:k + 1])
                    for oc in range(coutc):
                        ps = c1p.tile([128, nchunk], F32, tag="c1")
                        for kc in range(c1kc):
                            nc.tensor.matmul(
                                ps[:],
                                w1b[kc][:, oc * 128:(oc + 1) * 128],
                                unk[kc][:, c0:c0 + nchunk],
                                start=(kc == 0), stop=False,
                                skip_group_check=True)
                        for lb in range(cblk):
                            for k in range(3):
                                nc.tensor.matmul(
                                    ps[:, lb * 128:(lb + 1) * 128],
                                    gk[k][:, (c0 - g0) // 128 + lb,
                                          oc * 128:(oc + 1) * 128],
                                    dg[:, lb, k, :],
                                    start=False,
                                    stop=(lb == cblk - 1 and k == 2),
                                    skip_group_check=True)
                        nc.scalar.activation(
                            x2[oc][:, c0:c0 + nchunk], ps[:], AF.Relu,
                            bias=b1[oc][:], scale=1.0)
                    for oc in range(coutc):
                        ps2 = c2p.tile([128, nchunk], F32, tag="c2")
                        for kc in range(coutc):
                            nc.tensor.matmul(
                                ps2[:],
                                w2[kc][:, oc * 128:(oc + 1) * 128],
                                x2[kc][:, c0:c0 + nchunk],
                                start=(kc == 0), stop=(kc == coutc - 1))
                        nc.scalar.activation(
                            fo[oc][:, c0:c0 + nchunk], ps2[:], AF.Relu,
                            bias=b2[oc][:], scale=1.0)

        if os.environ.get("DEBUG_X2") and si == 1:
            for oc in range(coutc):
                nc.gpsimd.dma_start(
                    out_dram[:, 4096 + oc * n:4096 + (oc + 1) * n], x2[oc][:])
        if last:
            nc.sync.dma_start(out_dram[:], fo[0][:])
            return None
        return fo


# ---------------------------------------------------------------------------
# host-side prep
# ---------------------------------------------------------------------------

def _split_bf16(x):
    h = bf(x).astype(np.float32)
    l = bf(x.astype(np.float32) - h).astype(np.float32)
    return h, l


def _prep_shared(params):
    """Per-core-invariant inputs (weights, identity)."""
    im = {}
    for si, st in enumerate(STAGES):
        C2 = st["C2"]
        (W1, g1, be1, mu1, va1), (W2, g2, be2, mu2, va2) = params[3 - si]
        s1 = g1 / np.sqrt(va1 + BN_EPS)
        W1f = (W1 * s1[:, None]).astype(np.float32)
        bias1 = (be1 - mu1 * s1).astype(np.float32)
        s2 = g2 / np.sqrt(va2 + BN_EPS)
        W2f = (W2 * s2[:, None]).astype(np.float32)
        bias2 = (be2 - mu2 * s2).astype(np.float32)
        im[f"w1b{si}"] = np.ascontiguousarray(bf(W1f[:, C2:].T))
        im[f"b1{si}"] = np.ascontiguousarray(bias1[:, None])
        im[f"w2{si}"] = np.ascontiguousarray(bf(W2f.T))
        im[f"b2{si}"] = np.ascontiguousarray(bias2[:, None])
        if si > 0:
            im[f"w1a{si}"] = np.ascontiguousarray(bf(W1f[:, :C2].T))
        else:
            im["_W1a0"] = W1f[:, :C2]  # used per-core for gt0
    id3 = np.zeros((128, 384), np.float32)
    for kk in range(3):
        id3[:, kk * 128:(kk + 1) * 128] = np.eye(128)
    im["ident3"] = bf(id3)
    return im


def _prep_core(b, xyz, feats, shared):
    im = {k: v for k, v in shared.items() if not k.startswith("_")}
    for si, st in enumerate(STAGES):
        n, m = st["n"], st["m"]
        u = xyz[3 - si][b]      # (n, 3) unknown side
        kk = xyz[4 - si][b]     # (m, 3) known side
        uh, ul = _split_bf16(u)
        kh, kl = _split_bf16(kk)
        q = np.sum(kk.astype(np.float64) ** 2, -1)
        qh = bf(q).astype(np.float64)
        ql = (q - qh).astype(np.float32)
        lhsT = np.zeros((14, n), np.float32)
        lhsT[0:3] = uh.T
        lhsT[3:6] = ul.T
        lhsT[6:9] = uh.T
        lhsT[9:12] = ul.T
        lhsT[12] = 1.0
        lhsT[13] = 1.0
        rhs = np.zeros((14, m), np.float32)
        rhs[0:3] = 2.0 * kh.T
        rhs[3:6] = 2.0 * kh.T
        rhs[6:9] = 2.0 * kl.T
        rhs[9:12] = 2.0 * kl.T
        rhs[12] = -qh.astype(np.float32)
        rhs[13] = -ql
        im[f"lhsT{si}"] = bf(lhsT)
        im[f"rhs{si}"] = bf(rhs)
        usq = np.sum(u.astype(np.float32) ** 2, -1)
        im[f"usq{si}"] = np.ascontiguousarray(usq.reshape(n // 128, 128).T)
        im[f"unk{si}"] = np.ascontiguousarray(bf(feats[3 - si][b]))
        if si == 0:
            gt = feats[4][b].T.astype(np.float32) @ shared["_W1a0"].T
            im["gt0"] = np.ascontiguousarray(bf(gt))
    return im


_cache = {}


def kernel(**inputs):
    xyz = [np.asarray(inputs[f"xyz{l}"], np.float32) for l in range(5)]
    feats = [np.asarray(inputs[f"feat{l}"], np.float32) for l in range(5)]
    params = [[(np.asarray(W, np.float32), np.asarray(g, np.float32),
                np.asarray(bb, np.float32), np.asarray(mu, np.float32),
                np.asarray(v, np.float32)) for (W, g, bb, mu, v) in lp]
              for lp in inputs["params"]]

    if "nc" not in _cache:
        _cache["nc"] = build_module()
    nc = _cache["nc"]

    shared = _prep_shared(params)
    in_maps = [_prep_core(b, xyz, feats, shared) for b in range(B)]
    res = run_bass_kernel_spmd(nc, in_maps, core_ids=list(range(B)))
    out = np.stack([res.results[b]["out"] for b in range(B)], 0)
    return out.astype(np.float32)
